# revision 32
# baseline (speedup 1.0000x reference)
"""BOW classifier kernel for 8 Trainium2 NeuronCores.

Data-parallel over the batch dim (128 columns per core).  The embedding
mean-pool is reformulated as a dense count matmul instead of a per-row
gather: for each core the host builds A[v, b] = count of token v in column
b's valid prefix, restricted to the ~24-26k vocab rows the core actually
references.  The device computes pooled*len = A^T @ emb_used via
accumulating 128x128x300 fp16 matmuls on the tensor engine, fed by one
sequential full-bandwidth fp16 embedding stream plus a SBUF-resident count
matrix (two 4-bit counts packed per byte, unpacked per tile with a single
AND/SHIFT on the idle vector engine) -- no per-row DMA descriptors at all.
All MLP weights and the lengths ship in one packed aux DMA; trailing
stream tiles taper off so the tensor engine finishes almost with the last
DMA.  The MLP tail runs transposed (hT = W1^T @ pooled^T) so only the
300-wide pooled tensor is transposed, fc1/fc2 run in fp16 with biases
folded in as ones-row matmuls, and the [128, 2] result is transposed so
the store is two DMA records.
"""

import sys

import numpy as np

for _p in ("/opt/trn_rl_repo",):
    if _p not in sys.path:
        sys.path.insert(0, _p)

V, E, H, O = 50000, 300, 512, 2
S, B = 512, 1024
NCORES = 8
BS = B // NCORES   # 128 batch columns per core
EP = 304           # emb cols per chunk incl pad (608 B, 32-B aligned slices)
G = 24             # max chunks per DMA tile (line len must stay < 16 KB)
NBUF = 3           # rotating stream buffers
# aux-tensor layout (fp16 cols): 3x512 w1 chunks, 4 w2 chunks at 64-B slots,
# b2, lengths -- one DMA instead of ~1500 tiny records
AUX_W2 = [1536 + 32 * j for j in range(4)]
AUX_B2 = 1664
AUX_LEN = 1696
AUXW = 1728


def _tile_plan(n, taper=False):
    """Full G-chunk tiles; optionally taper the trailing tiles so the PE
    catch-up after the last DMA is short."""
    tiles = []
    while n > G:
        tiles.append(G)
        n -= G
    if taper and n > 8:
        tiles += [n - 4, 4]
    elif n:
        tiles.append(n)
    return tiles


def _plan(nct, mode):
    """Per-tile (chunk_start, g, half) schedule.  In nibble mode the stream
    is split at Hc: the packed byte holds chunk c in the low nibble and
    chunk c+Hc in the high nibble."""
    if mode == "nib":
        hc = -(-nct // 2)
        t1 = _tile_plan(hc)
        t2 = _tile_plan(nct - hc, taper=True)
        plan = []
        c = 0
        for g in t1:
            plan.append((c, g, 0))
            c += g
        for g in t2:
            plan.append((c, g, 1))
            c += g
        return plan, hc
    plan = []
    c = 0
    for g in _tile_plan(nct, taper=True):
        plan.append((c, g, 0))
        c += g
    return plan, nct


def _build_nc(nct, mode):
    from contextlib import ExitStack

    import concourse.tile as tile
    from concourse import bacc, mybir
    from concourse.masks import make_identity

    f16, f32, u8 = mybir.dt.float16, mybir.dt.float32, mybir.dt.uint8

    plan, hc = _plan(nct, mode)
    ntiles = len(plan)

    nc = bacc.Bacc(None, target_bir_lowering=False)
    a_d = nc.declare_dram_parameter(
        "aw", [BS, hc * 128], f16 if mode == "f16" else u8, isOutput=False)
    e_d = nc.declare_dram_parameter("ew", [ntiles * BS, G * EP], f16,
                                    isOutput=False)
    aux_d = nc.declare_dram_parameter("aux", [BS, AUXW], f16, isOutput=False)
    out_d = nc.declare_dram_parameter("out", [O, BS], f32, isOutput=True)

    with tile.TileContext(nc) as tc, ExitStack() as ctx:
        sb = ctx.enter_context(tc.tile_pool(name="sb", bufs=1))
        st = ctx.enter_context(tc.tile_pool(name="st", bufs=NBUF))
        ps = ctx.enter_context(tc.tile_pool(name="ps", bufs=1, space="PSUM"))
        ps2 = ctx.enter_context(tc.tile_pool(name="ps2", bufs=2, space="PSUM"))

        aux = sb.tile([BS, AUXW], f16, tag="aux")
        nc.sync.dma_start(out=aux[:], in_=aux_d[:])

        # counts resident in SBUF, loaded in two big sequential bursts
        a_sb = sb.tile([BS, hc * 128], f16 if mode == "f16" else u8, tag="aw")
        half = (hc // 2) * 128
        nc.sync.dma_start(out=a_sb[:, 0:half], in_=a_d[:, 0:half])
        nc.sync.dma_start(out=a_sb[:, half:], in_=a_d[:, half:])

        # pooled*len accumulates over all vocab chunks in one PSUM bank
        hp = ps.tile([BS, E], f32, tag="hp", space="PSUM")
        for t, (c0, g, hi) in enumerate(plan):
            et = st.tile([BS, g * EP], f16, tag="ew")
            nc.sync.dma_start(out=et[:], in_=e_d[t * BS:t * BS + BS, 0:g * EP])
            src0 = (c0 - hc if hi else c0) * 128
            if mode == "f16":
                af = a_sb[:, src0:src0 + g * 128]
            else:
                af = st.tile([BS, g * 128], f16, tag="af")
                if mode == "u8":
                    nc.vector.tensor_copy(
                        out=af[:], in_=a_sb[:, src0:src0 + g * 128])
                else:
                    # bitVec ops cannot cast: extract u8->u8, then cast
                    a4 = st.tile([BS, g * 128], u8, tag="a4")
                    nc.vector.tensor_scalar(
                        out=a4[:], in0=a_sb[:, src0:src0 + g * 128],
                        scalar1=4 if hi else 15, scalar2=None,
                        op0=(mybir.AluOpType.logical_shift_right if hi
                             else mybir.AluOpType.bitwise_and),
                    )
                    nc.vector.tensor_copy(out=af[:], in_=a4[:])
            for k in range(g):
                nc.tensor.matmul(
                    out=hp[:],
                    lhsT=af[:, k * 128:(k + 1) * 128],
                    rhs=et[:, k * EP:k * EP + E],
                    start=(c0 + k == 0),
                    stop=(c0 + k == nct - 1),
                )

        # pooled = hp / len  (f32, then transposed+cast to f16 for the MLP)
        lenf = sb.tile([BS, 1], f32, tag="lenf")
        nc.vector.tensor_copy(out=lenf[:], in_=aux[:, AUX_LEN:AUX_LEN + 1])
        recip = sb.tile([BS, 1], f32, tag="recip")
        nc.vector.reciprocal(recip[:], lenf[:])
        pooled = sb.tile([BS, E], f32, tag="pooled")
        nc.vector.tensor_scalar(
            out=pooled[:], in0=hp[:], scalar1=recip[:, 0:1], scalar2=None,
            op0=mybir.AluOpType.mult,
        )

        # pooled^T chunks (f16), chunk 2 padded with a ones row (fc1 bias)
        ident = sb.tile([128, 128], f32, tag="ident")
        make_identity(nc, ident[:])
        pT = []
        for c, (c0, c1) in enumerate([(0, 128), (128, 256), (256, E)]):
            w = c1 - c0
            pt = ps2.tile([w, 128], f32, tag="tr", space="PSUM")
            nc.tensor.transpose(out=pt[:], in_=pooled[:, c0:c1], identity=ident[:])
            rows = w + 1 if c == 2 else w
            lt = sb.tile([rows, 128], f16, tag=f"pT{c}")
            if c == 2:
                nc.vector.memset(lt[:], 1.0)
            nc.vector.tensor_copy(out=lt[0:w, :], in_=pt[:])
            pT.append(lt)

        # fc1 transposed: hT_j = W1b[:, j]^T @ pooled^T -> relu -> f16
        # (relu split across the scalar and vector engines)
        crows = [(0, 128), (128, 256), (256, E + 1)]
        hT = []
        for j in range(4):
            htp = ps2.tile([128, BS], f32, tag="htp", space="PSUM")
            for c, (r0, r1) in enumerate(crows):
                nc.tensor.matmul(
                    out=htp[:],
                    lhsT=aux[0:r1 - r0, c * 512 + j * 128:c * 512 + (j + 1) * 128],
                    rhs=pT[c][:], start=(c == 0), stop=(c == 2),
                )
            ht = sb.tile([128, BS], f16, tag=f"hT{j}")
            if j % 2:
                nc.scalar.activation(out=ht[:], in_=htp[:],
                                     func=mybir.ActivationFunctionType.Relu)
            else:
                nc.vector.tensor_scalar(
                    out=ht[:], in0=htp[:], scalar1=0.0, scalar2=None,
                    op0=mybir.AluOpType.max,
                )
            hT.append(ht)

        # fc2: out = h @ W2 + b2 (hT_j is already the lhsT layout)
        ones1 = sb.tile([1, BS], f16, tag="ones1")
        nc.vector.memset(ones1[:], 1.0)
        op_ = ps.tile([BS, O], f32, tag="op", space="PSUM")
        for j in range(4):
            nc.tensor.matmul(out=op_[:], lhsT=hT[j][:],
                             rhs=aux[:, AUX_W2[j]:AUX_W2[j] + O],
                             start=(j == 0), stop=False)
        nc.tensor.matmul(out=op_[:], lhsT=ones1[:],
                         rhs=aux[0:1, AUX_B2:AUX_B2 + O],
                         start=False, stop=True)
        # transpose the [128, 2] result to [2, 128] so the store is 2 records
        out_sb = sb.tile([BS, O], f32, tag="osb")
        nc.vector.tensor_copy(out=out_sb[:], in_=op_[:])
        otp = ps2.tile([O, BS], f32, tag="otp", space="PSUM")
        nc.tensor.transpose(out=otp[:], in_=out_sb[:], identity=ident[:])
        oT = sb.tile([O, BS], f32, tag="oT")
        nc.vector.tensor_copy(out=oT[:], in_=otp[:])
        nc.sync.dma_start(out=out_d[:], in_=oT[:])

    nc.finalize()
    return nc


def _prep_in_maps(text, lengths, emb_table, W1, b1, W2, b2):
    text = np.asarray(text).astype(np.int64)        # [S, B]
    lengths = np.asarray(lengths).astype(np.int64)  # [B]
    emb = np.asarray(emb_table, np.float32)
    w1b = np.vstack([np.asarray(W1, np.float32),
                     np.asarray(b1, np.float32)[None, :]]).astype(np.float16)
    w2 = np.asarray(W2, np.float32).astype(np.float16)
    b2f = np.asarray(b2, np.float32).astype(np.float16)

    # assign columns to cores greedily to minimize the max distinct-token
    # count per core (the slowest core's count sets every core's stream
    # length); kernel output is unpermuted on the host afterwards
    import os
    svec = np.arange(S)[:, None]
    col_tokens = [np.unique(text[:lengths[b], b]) for b in range(B)]
    order = np.argsort([-len(t) for t in col_tokens])
    if os.environ.get("KERNEL_NO_REBALANCE"):
        col_perm = np.arange(B)
        order = []
    seen = np.zeros((NCORES, V), bool)
    counts = [0] * NCORES
    dist = [0] * NCORES
    assign = [[] for _ in range(NCORES)]
    for b in order:
        toks = col_tokens[b]
        best, binc = None, None
        for i in range(NCORES):
            if counts[i] >= BS:
                continue
            inc = int(np.count_nonzero(~seen[i, toks]))
            # tie-break toward the emptier core to keep loads even
            key = (dist[i] + inc, counts[i])
            if best is None or key < bkey:
                best, binc, bkey = i, inc, key
        assign[best].append(b)
        counts[best] += 1
        dist[best] += binc
        seen[best, col_tokens[b]] = True
    if not os.environ.get("KERNEL_NO_REBALANCE"):
        col_perm = np.concatenate([np.sort(np.array(a, np.int64))
                                   for a in assign])

    colid = np.broadcast_to(np.arange(BS)[None, :], (S, BS))
    per_core = []
    cmax = 0
    for i in range(NCORES):
        cols = col_perm[i * BS:(i + 1) * BS]
        t_sh = text[:, cols]
        l_sh = lengths[cols]
        mask = svec < l_sh[None, :]
        used, inv = np.unique(t_sh[mask], return_inverse=True)
        cnt = np.zeros((len(used), BS), np.float32)
        np.add.at(cnt, (inv, colid[mask]), 1.0)
        cmax = max(cmax, cnt.max())
        per_core.append((used, cnt, l_sh))

    mode = "nib" if cmax <= 15 else ("u8" if cmax <= 255 else "f16")
    nmax = max(len(u) for u, _, _ in per_core)
    nct = -(-nmax // 128)
    plan, hc = _plan(nct, mode)
    ntiles = len(plan)

    in_maps = []
    for used, cnt, l_sh in per_core:
        n = len(used)
        if mode == "nib":
            a_pad = np.zeros((2 * hc * 128, BS), np.uint8)
            a_pad[:n] = cnt
            a_full = a_pad[:hc * 128] | (a_pad[hc * 128:] << 4)
        else:
            a_full = np.zeros((hc * 128, BS),
                              np.uint8 if mode == "u8" else np.float16)
            a_full[:n] = cnt
        a_w = (a_full.reshape(hc, 128, BS)
               .transpose(1, 0, 2).reshape(BS, hc * 128))
        e_full = np.zeros((nct * 128, E), np.float16)
        e_full[:n] = emb[used]
        e_w = np.zeros((ntiles * BS, G * EP), np.float16)
        for t, (c0, g, _) in enumerate(plan):
            ech = e_full[c0 * 128:(c0 + g) * 128].reshape(g, 128, E)
            e_w[t * BS:(t + 1) * BS, :g * EP] = (
                np.pad(ech, ((0, 0), (0, 0), (0, EP - E)))
                .transpose(1, 0, 2).reshape(BS, g * EP))
        aux = np.zeros((BS, AUXW), np.float16)
        for c, (r0, r1) in enumerate([(0, 128), (128, 256), (256, E + 1)]):
            aux[0:r1 - r0, c * 512:(c + 1) * 512] = w1b[r0:r1]
        for j in range(4):
            aux[:, AUX_W2[j]:AUX_W2[j] + O] = w2[j * 128:(j + 1) * 128]
        aux[0, AUX_B2:AUX_B2 + O] = b2f
        aux[:, AUX_LEN] = l_sh.astype(np.float16)  # lengths <= 512, exact
        in_maps.append({
            "aw": np.ascontiguousarray(a_w),
            "ew": np.ascontiguousarray(e_w),
            "aux": aux,
        })
    return in_maps, nct, mode, col_perm


def _run(inputs, trace=False):
    from concourse.bass_utils import run_bass_kernel_spmd

    in_maps, nct, mode, col_perm = _prep_in_maps(**inputs)
    nc = _build_nc(nct, mode)
    res = run_bass_kernel_spmd(nc, in_maps, list(range(NCORES)), trace=trace)
    perm_out = np.concatenate(
        [res.results[i]["out"].T for i in range(NCORES)], axis=0)
    out = np.empty_like(perm_out)
    out[col_perm] = perm_out
    return out.astype(np.float32), res


def kernel(**inputs):
    out, _ = _run(inputs, trace=False)
    return out


# revision 33
# speedup vs baseline: 1.0584x; 1.0584x over previous
"""BOW classifier kernel for 8 Trainium2 NeuronCores.

Data-parallel over the batch dim (128 columns per core).  The embedding
mean-pool is reformulated as a dense count matmul instead of a per-row
gather: for each core the host builds A[v, b] = count of token v in column
b's valid prefix, restricted to the ~24-26k vocab rows the core actually
references.  The device computes pooled*len = A^T @ emb_used via
accumulating 128x128x300 fp16 matmuls on the tensor engine, fed by one
sequential full-bandwidth fp16 embedding stream plus a SBUF-resident count
matrix (two 4-bit counts packed per byte, unpacked per tile with a single
AND/SHIFT on the idle vector engine) -- no per-row DMA descriptors at all.
All MLP weights and the lengths ship in one packed aux DMA; trailing
stream tiles taper off so the tensor engine finishes almost with the last
DMA.  The MLP tail runs transposed (hT = W1^T @ pooled^T) so only the
300-wide pooled tensor is transposed, fc1/fc2 run in fp16 with biases
folded in as ones-row matmuls, and the [128, 2] result is transposed so
the store is two DMA records.
"""

import sys

import numpy as np

for _p in ("/opt/trn_rl_repo",):
    if _p not in sys.path:
        sys.path.insert(0, _p)

V, E, H, O = 50000, 300, 512, 2
S, B = 512, 1024
NCORES = 8
BS = B // NCORES   # 128 batch columns per core
EP = 304           # emb cols per chunk incl pad (608 B, 32-B aligned slices)
G = 26             # max chunks per DMA tile (line len must stay < 16 KB)
NBUF = 4           # rotating stream buffers
# aux-tensor layout (fp16 cols): 3x512 w1 chunks, 4 w2 chunks at 64-B slots,
# b2, lengths -- one DMA instead of ~1500 tiny records
AUX_W2 = [1536 + 32 * j for j in range(4)]
AUX_B2 = 1664
AUX_LEN = 1696
AUXW = 1728


def _tile_plan(n, taper=False):
    """Full G-chunk tiles; optionally taper the trailing tiles so the PE
    catch-up after the last DMA is short."""
    tiles = []
    while n > G:
        tiles.append(G)
        n -= G
    if taper and n > 8:
        tiles += [n - 4, 4]
    elif n:
        tiles.append(n)
    return tiles


def _plan(nct, mode):
    """Per-tile (chunk_start, g, half) schedule.  In nibble mode the stream
    is split at Hc: the packed byte holds chunk c in the low nibble and
    chunk c+Hc in the high nibble."""
    if mode == "nib":
        hc = -(-nct // 2)
        t1 = _tile_plan(hc)
        t2 = _tile_plan(nct - hc, taper=True)
        plan = []
        c = 0
        for g in t1:
            plan.append((c, g, 0))
            c += g
        for g in t2:
            plan.append((c, g, 1))
            c += g
        return plan, hc
    plan = []
    c = 0
    for g in _tile_plan(nct, taper=True):
        plan.append((c, g, 0))
        c += g
    return plan, nct


def _build_nc(nct, mode):
    from contextlib import ExitStack

    import concourse.tile as tile
    from concourse import bacc, mybir
    from concourse.masks import make_identity

    f16, f32, u8 = mybir.dt.float16, mybir.dt.float32, mybir.dt.uint8

    plan, hc = _plan(nct, mode)
    ntiles = len(plan)

    nc = bacc.Bacc(None, target_bir_lowering=False)
    a_d = nc.declare_dram_parameter(
        "aw", [BS, hc * 128], f16 if mode == "f16" else u8, isOutput=False)
    e_d = nc.declare_dram_parameter("ew", [ntiles * BS, G * EP], f16,
                                    isOutput=False)
    aux_d = nc.declare_dram_parameter("aux", [BS, AUXW], f16, isOutput=False)
    out_d = nc.declare_dram_parameter("out", [O, BS], f32, isOutput=True)

    with tile.TileContext(nc) as tc, ExitStack() as ctx:
        sb = ctx.enter_context(tc.tile_pool(name="sb", bufs=1))
        st = ctx.enter_context(tc.tile_pool(name="st", bufs=NBUF))
        ps = ctx.enter_context(tc.tile_pool(name="ps", bufs=1, space="PSUM"))
        ps2 = ctx.enter_context(tc.tile_pool(name="ps2", bufs=2, space="PSUM"))

        aux = sb.tile([BS, AUXW], f16, tag="aux")
        nc.sync.dma_start(out=aux[:], in_=aux_d[:])

        # counts resident in SBUF, loaded in two big sequential bursts
        a_sb = sb.tile([BS, hc * 128], f16 if mode == "f16" else u8, tag="aw")
        half = (hc // 2) * 128
        nc.sync.dma_start(out=a_sb[:, 0:half], in_=a_d[:, 0:half])
        nc.sync.dma_start(out=a_sb[:, half:], in_=a_d[:, half:])

        # pooled*len accumulates over all vocab chunks in one PSUM bank
        hp = ps.tile([BS, E], f32, tag="hp", space="PSUM")
        for t, (c0, g, hi) in enumerate(plan):
            et = st.tile([BS, g * EP], f16, tag="ew")
            nc.sync.dma_start(out=et[:], in_=e_d[t * BS:t * BS + BS, 0:g * EP])
            src0 = (c0 - hc if hi else c0) * 128
            if mode == "f16":
                af = a_sb[:, src0:src0 + g * 128]
            else:
                af = st.tile([BS, g * 128], f16, tag="af")
                if mode == "u8":
                    nc.vector.tensor_copy(
                        out=af[:], in_=a_sb[:, src0:src0 + g * 128])
                else:
                    # bitVec ops cannot cast: extract u8->u8, then cast
                    a4 = st.tile([BS, g * 128], u8, tag="a4")
                    nc.vector.tensor_scalar(
                        out=a4[:], in0=a_sb[:, src0:src0 + g * 128],
                        scalar1=4 if hi else 15, scalar2=None,
                        op0=(mybir.AluOpType.logical_shift_right if hi
                             else mybir.AluOpType.bitwise_and),
                    )
                    nc.vector.tensor_copy(out=af[:], in_=a4[:])
            for k in range(g):
                nc.tensor.matmul(
                    out=hp[:],
                    lhsT=af[:, k * 128:(k + 1) * 128],
                    rhs=et[:, k * EP:k * EP + E],
                    start=(c0 + k == 0),
                    stop=(c0 + k == nct - 1),
                )

        # pooled = hp / len  (f32, then transposed+cast to f16 for the MLP)
        lenf = sb.tile([BS, 1], f32, tag="lenf")
        nc.vector.tensor_copy(out=lenf[:], in_=aux[:, AUX_LEN:AUX_LEN + 1])
        recip = sb.tile([BS, 1], f32, tag="recip")
        nc.vector.reciprocal(recip[:], lenf[:])
        pooled = sb.tile([BS, E], f32, tag="pooled")
        nc.vector.tensor_scalar(
            out=pooled[:], in0=hp[:], scalar1=recip[:, 0:1], scalar2=None,
            op0=mybir.AluOpType.mult,
        )

        # pooled^T chunks (f16), chunk 2 padded with a ones row (fc1 bias)
        ident = sb.tile([128, 128], f32, tag="ident")
        make_identity(nc, ident[:])
        pT = []
        for c, (c0, c1) in enumerate([(0, 128), (128, 256), (256, E)]):
            w = c1 - c0
            pt = ps2.tile([w, 128], f32, tag="tr", space="PSUM")
            nc.tensor.transpose(out=pt[:], in_=pooled[:, c0:c1], identity=ident[:])
            rows = w + 1 if c == 2 else w
            lt = sb.tile([rows, 128], f16, tag=f"pT{c}")
            if c == 2:
                nc.vector.memset(lt[:], 1.0)
            nc.vector.tensor_copy(out=lt[0:w, :], in_=pt[:])
            pT.append(lt)

        # fc1 transposed: hT_j = W1b[:, j]^T @ pooled^T -> relu -> f16
        # (relu split across the scalar and vector engines)
        crows = [(0, 128), (128, 256), (256, E + 1)]
        hT = []
        for j in range(4):
            htp = ps2.tile([128, BS], f32, tag="htp", space="PSUM")
            for c, (r0, r1) in enumerate(crows):
                nc.tensor.matmul(
                    out=htp[:],
                    lhsT=aux[0:r1 - r0, c * 512 + j * 128:c * 512 + (j + 1) * 128],
                    rhs=pT[c][:], start=(c == 0), stop=(c == 2),
                )
            ht = sb.tile([128, BS], f16, tag=f"hT{j}")
            if j % 2:
                nc.scalar.activation(out=ht[:], in_=htp[:],
                                     func=mybir.ActivationFunctionType.Relu)
            else:
                nc.vector.tensor_scalar(
                    out=ht[:], in0=htp[:], scalar1=0.0, scalar2=None,
                    op0=mybir.AluOpType.max,
                )
            hT.append(ht)

        # fc2: out = h @ W2 + b2 (hT_j is already the lhsT layout)
        ones1 = sb.tile([1, BS], f16, tag="ones1")
        nc.vector.memset(ones1[:], 1.0)
        op_ = ps.tile([BS, O], f32, tag="op", space="PSUM")
        for j in range(4):
            nc.tensor.matmul(out=op_[:], lhsT=hT[j][:],
                             rhs=aux[:, AUX_W2[j]:AUX_W2[j] + O],
                             start=(j == 0), stop=False)
        nc.tensor.matmul(out=op_[:], lhsT=ones1[:],
                         rhs=aux[0:1, AUX_B2:AUX_B2 + O],
                         start=False, stop=True)
        # transpose the [128, 2] result to [2, 128] so the store is 2 records
        out_sb = sb.tile([BS, O], f32, tag="osb")
        nc.vector.tensor_copy(out=out_sb[:], in_=op_[:])
        otp = ps2.tile([O, BS], f32, tag="otp", space="PSUM")
        nc.tensor.transpose(out=otp[:], in_=out_sb[:], identity=ident[:])
        oT = sb.tile([O, BS], f32, tag="oT")
        nc.vector.tensor_copy(out=oT[:], in_=otp[:])
        nc.sync.dma_start(out=out_d[:], in_=oT[:])

    nc.finalize()
    return nc


def _prep_in_maps(text, lengths, emb_table, W1, b1, W2, b2):
    text = np.asarray(text).astype(np.int64)        # [S, B]
    lengths = np.asarray(lengths).astype(np.int64)  # [B]
    emb = np.asarray(emb_table, np.float32)
    w1b = np.vstack([np.asarray(W1, np.float32),
                     np.asarray(b1, np.float32)[None, :]]).astype(np.float16)
    w2 = np.asarray(W2, np.float32).astype(np.float16)
    b2f = np.asarray(b2, np.float32).astype(np.float16)

    # assign columns to cores greedily to minimize the max distinct-token
    # count per core (the slowest core's count sets every core's stream
    # length); kernel output is unpermuted on the host afterwards
    import os
    svec = np.arange(S)[:, None]
    col_tokens = [np.unique(text[:lengths[b], b]) for b in range(B)]
    order = np.argsort([-len(t) for t in col_tokens])
    if os.environ.get("KERNEL_NO_REBALANCE"):
        col_perm = np.arange(B)
        order = []
    seen = np.zeros((NCORES, V), bool)
    counts = [0] * NCORES
    dist = [0] * NCORES
    assign = [[] for _ in range(NCORES)]
    for b in order:
        toks = col_tokens[b]
        best, binc = None, None
        for i in range(NCORES):
            if counts[i] >= BS:
                continue
            inc = int(np.count_nonzero(~seen[i, toks]))
            # tie-break toward the emptier core to keep loads even
            key = (dist[i] + inc, counts[i])
            if best is None or key < bkey:
                best, binc, bkey = i, inc, key
        assign[best].append(b)
        counts[best] += 1
        dist[best] += binc
        seen[best, col_tokens[b]] = True
    if not os.environ.get("KERNEL_NO_REBALANCE"):
        col_perm = np.concatenate([np.sort(np.array(a, np.int64))
                                   for a in assign])

    colid = np.broadcast_to(np.arange(BS)[None, :], (S, BS))
    per_core = []
    cmax = 0
    for i in range(NCORES):
        cols = col_perm[i * BS:(i + 1) * BS]
        t_sh = text[:, cols]
        l_sh = lengths[cols]
        mask = svec < l_sh[None, :]
        used, inv = np.unique(t_sh[mask], return_inverse=True)
        cnt = np.zeros((len(used), BS), np.float32)
        np.add.at(cnt, (inv, colid[mask]), 1.0)
        cmax = max(cmax, cnt.max())
        per_core.append((used, cnt, l_sh))

    mode = "nib" if cmax <= 15 else ("u8" if cmax <= 255 else "f16")
    nmax = max(len(u) for u, _, _ in per_core)
    nct = -(-nmax // 128)
    plan, hc = _plan(nct, mode)
    ntiles = len(plan)

    in_maps = []
    for used, cnt, l_sh in per_core:
        n = len(used)
        if mode == "nib":
            a_pad = np.zeros((2 * hc * 128, BS), np.uint8)
            a_pad[:n] = cnt
            a_full = a_pad[:hc * 128] | (a_pad[hc * 128:] << 4)
        else:
            a_full = np.zeros((hc * 128, BS),
                              np.uint8 if mode == "u8" else np.float16)
            a_full[:n] = cnt
        a_w = (a_full.reshape(hc, 128, BS)
               .transpose(1, 0, 2).reshape(BS, hc * 128))
        e_full = np.zeros((nct * 128, E), np.float16)
        e_full[:n] = emb[used]
        e_w = np.zeros((ntiles * BS, G * EP), np.float16)
        for t, (c0, g, _) in enumerate(plan):
            ech = e_full[c0 * 128:(c0 + g) * 128].reshape(g, 128, E)
            e_w[t * BS:(t + 1) * BS, :g * EP] = (
                np.pad(ech, ((0, 0), (0, 0), (0, EP - E)))
                .transpose(1, 0, 2).reshape(BS, g * EP))
        aux = np.zeros((BS, AUXW), np.float16)
        for c, (r0, r1) in enumerate([(0, 128), (128, 256), (256, E + 1)]):
            aux[0:r1 - r0, c * 512:(c + 1) * 512] = w1b[r0:r1]
        for j in range(4):
            aux[:, AUX_W2[j]:AUX_W2[j] + O] = w2[j * 128:(j + 1) * 128]
        aux[0, AUX_B2:AUX_B2 + O] = b2f
        aux[:, AUX_LEN] = l_sh.astype(np.float16)  # lengths <= 512, exact
        in_maps.append({
            "aw": np.ascontiguousarray(a_w),
            "ew": np.ascontiguousarray(e_w),
            "aux": aux,
        })
    return in_maps, nct, mode, col_perm


def _run(inputs, trace=False):
    from concourse.bass_utils import run_bass_kernel_spmd

    in_maps, nct, mode, col_perm = _prep_in_maps(**inputs)
    nc = _build_nc(nct, mode)
    res = run_bass_kernel_spmd(nc, in_maps, list(range(NCORES)), trace=trace)
    perm_out = np.concatenate(
        [res.results[i]["out"].T for i in range(NCORES)], axis=0)
    out = np.empty_like(perm_out)
    out[col_perm] = perm_out
    return out.astype(np.float32), res


def kernel(**inputs):
    out, _ = _run(inputs, trace=False)
    return out


# revision 34
# speedup vs baseline: 1.0791x; 1.0196x over previous
"""BOW classifier kernel for 8 Trainium2 NeuronCores.

Data-parallel over the batch dim (128 columns per core).  The embedding
mean-pool is reformulated as a dense count matmul instead of a per-row
gather: for each core the host builds A[v, b] = count of token v in column
b's valid prefix, restricted to the ~24-26k vocab rows the core actually
references.  The device computes pooled*len = A^T @ emb_used via
accumulating 128x128x300 fp16 matmuls on the tensor engine, fed by one
sequential full-bandwidth fp16 embedding stream plus a SBUF-resident count
matrix (two 4-bit counts packed per byte, unpacked per tile with a single
AND/SHIFT on the idle vector engine) -- no per-row DMA descriptors at all.
All MLP weights and the lengths ship in one packed aux DMA; trailing
stream tiles taper off so the tensor engine finishes almost with the last
DMA.  The MLP tail runs transposed (hT = W1^T @ pooled^T) so only the
300-wide pooled tensor is transposed, fc1/fc2 run in fp16 with biases
folded in as ones-row matmuls, and the [128, 2] result is transposed so
the store is two DMA records.
"""

import sys

import numpy as np

for _p in ("/opt/trn_rl_repo",):
    if _p not in sys.path:
        sys.path.insert(0, _p)

V, E, H, O = 50000, 300, 512, 2
S, B = 512, 1024
NCORES = 8
BS = B // NCORES   # 128 batch columns per core
EP = 304           # emb cols per chunk incl pad (608 B, 32-B aligned slices)
G = 26             # max chunks per DMA tile (line len must stay < 16 KB)
NBUF = 5           # rotating stream buffers
SUB = 13           # chunks per convert sub-block (finer DVE->PE chase)
# aux-tensor layout (fp16 cols): 3x512 w1 chunks, 4 w2 chunks at 64-B slots,
# b2, lengths -- one DMA instead of ~1500 tiny records
AUX_W2 = [1536 + 32 * j for j in range(4)]
AUX_B2 = 1664
AUX_LEN = 1696
AUXW = 1728


def _tile_plan(n, taper=False):
    """Full G-chunk tiles; optionally taper the trailing tiles so the PE
    catch-up after the last DMA is short."""
    tiles = []
    while n > G:
        tiles.append(G)
        n -= G
    if taper and n > 8:
        tiles += [n - 4, 4]
    elif n:
        tiles.append(n)
    return tiles


def _plan(nct, mode):
    """Per-tile (chunk_start, g, half) schedule.  In nibble mode the stream
    is split at Hc: the packed byte holds chunk c in the low nibble and
    chunk c+Hc in the high nibble."""
    if mode == "nib":
        hc = -(-nct // 2)
        t1 = _tile_plan(hc)
        t2 = _tile_plan(nct - hc, taper=True)
        plan = []
        c = 0
        for g in t1:
            plan.append((c, g, 0))
            c += g
        for g in t2:
            plan.append((c, g, 1))
            c += g
        return plan, hc
    plan = []
    c = 0
    for g in _tile_plan(nct, taper=True):
        plan.append((c, g, 0))
        c += g
    return plan, nct


def _build_nc(nct, mode):
    from contextlib import ExitStack

    import concourse.tile as tile
    from concourse import bacc, mybir
    from concourse.masks import make_identity

    f16, f32, u8 = mybir.dt.float16, mybir.dt.float32, mybir.dt.uint8

    plan, hc = _plan(nct, mode)
    ntiles = len(plan)

    nc = bacc.Bacc(None, target_bir_lowering=False)
    a_d = nc.declare_dram_parameter(
        "aw", [BS, hc * 128], f16 if mode == "f16" else u8, isOutput=False)
    e_d = nc.declare_dram_parameter("ew", [ntiles * BS, G * EP], f16,
                                    isOutput=False)
    aux_d = nc.declare_dram_parameter("aux", [BS, AUXW], f16, isOutput=False)
    out_d = nc.declare_dram_parameter("out", [O, BS], f32, isOutput=True)

    with tile.TileContext(nc) as tc, ExitStack() as ctx:
        sb = ctx.enter_context(tc.tile_pool(name="sb", bufs=1))
        st = ctx.enter_context(tc.tile_pool(name="st", bufs=NBUF))
        ps = ctx.enter_context(tc.tile_pool(name="ps", bufs=1, space="PSUM"))
        ps2 = ctx.enter_context(tc.tile_pool(name="ps2", bufs=2, space="PSUM"))

        aux = sb.tile([BS, AUXW], f16, tag="aux")
        nc.sync.dma_start(out=aux[:], in_=aux_d[:])

        # counts resident in SBUF, loaded in two big sequential bursts
        a_sb = sb.tile([BS, hc * 128], f16 if mode == "f16" else u8, tag="aw")
        half = (hc // 2) * 128
        nc.sync.dma_start(out=a_sb[:, 0:half], in_=a_d[:, 0:half])
        nc.sync.dma_start(out=a_sb[:, half:], in_=a_d[:, half:])

        # pooled*len accumulates over all vocab chunks in one PSUM bank
        hp = ps.tile([BS, E], f32, tag="hp", space="PSUM")
        for t, (c0, g, hi) in enumerate(plan):
            et = st.tile([BS, g * EP], f16, tag="ew")
            nc.sync.dma_start(out=et[:], in_=e_d[t * BS:t * BS + BS, 0:g * EP])
            src0 = (c0 - hc if hi else c0) * 128
            for s0 in range(0, g, SUB):
                w = min(SUB, g - s0)
                sb0 = src0 + s0 * 128
                if mode == "f16":
                    afs = a_sb[:, sb0:sb0 + w * 128]
                elif mode == "u8":
                    afs = st.tile([BS, w * 128], f16, tag="af")
                    nc.vector.tensor_copy(
                        out=afs[:], in_=a_sb[:, sb0:sb0 + w * 128])
                else:
                    # bitVec ops cannot cast: extract u8->u8, then cast
                    a4 = st.tile([BS, w * 128], u8, tag="a4")
                    nc.vector.tensor_scalar(
                        out=a4[:], in0=a_sb[:, sb0:sb0 + w * 128],
                        scalar1=4 if hi else 15, scalar2=None,
                        op0=(mybir.AluOpType.logical_shift_right if hi
                             else mybir.AluOpType.bitwise_and),
                    )
                    afs = st.tile([BS, w * 128], f16, tag="af")
                    nc.vector.tensor_copy(out=afs[:], in_=a4[:])
                for k in range(w):
                    kk = s0 + k
                    nc.tensor.matmul(
                        out=hp[:],
                        lhsT=afs[:, k * 128:(k + 1) * 128],
                        rhs=et[:, kk * EP:kk * EP + E],
                        start=(c0 + kk == 0),
                        stop=(c0 + kk == nct - 1),
                    )

        # pooled = hp / len  (f32, then transposed+cast to f16 for the MLP)
        lenf = sb.tile([BS, 1], f32, tag="lenf")
        nc.vector.tensor_copy(out=lenf[:], in_=aux[:, AUX_LEN:AUX_LEN + 1])
        recip = sb.tile([BS, 1], f32, tag="recip")
        nc.vector.reciprocal(recip[:], lenf[:])
        pooled = sb.tile([BS, E], f32, tag="pooled")
        nc.vector.tensor_scalar(
            out=pooled[:], in0=hp[:], scalar1=recip[:, 0:1], scalar2=None,
            op0=mybir.AluOpType.mult,
        )

        # pooled^T chunks (f16), chunk 2 padded with a ones row (fc1 bias)
        ident = sb.tile([128, 128], f32, tag="ident")
        make_identity(nc, ident[:])
        pT = []
        for c, (c0, c1) in enumerate([(0, 128), (128, 256), (256, E)]):
            w = c1 - c0
            pt = ps2.tile([w, 128], f32, tag="tr", space="PSUM")
            nc.tensor.transpose(out=pt[:], in_=pooled[:, c0:c1], identity=ident[:])
            rows = w + 1 if c == 2 else w
            lt = sb.tile([rows, 128], f16, tag=f"pT{c}")
            if c == 2:
                nc.vector.memset(lt[:], 1.0)
            nc.vector.tensor_copy(out=lt[0:w, :], in_=pt[:])
            pT.append(lt)

        # fc1 transposed: hT_j = W1b[:, j]^T @ pooled^T -> relu -> f16
        # (relu split across the scalar and vector engines)
        crows = [(0, 128), (128, 256), (256, E + 1)]
        hT = []
        for j in range(4):
            htp = ps2.tile([128, BS], f32, tag="htp", space="PSUM")
            for c, (r0, r1) in enumerate(crows):
                nc.tensor.matmul(
                    out=htp[:],
                    lhsT=aux[0:r1 - r0, c * 512 + j * 128:c * 512 + (j + 1) * 128],
                    rhs=pT[c][:], start=(c == 0), stop=(c == 2),
                )
            ht = sb.tile([128, BS], f16, tag=f"hT{j}")
            if j % 2:
                nc.scalar.activation(out=ht[:], in_=htp[:],
                                     func=mybir.ActivationFunctionType.Relu)
            else:
                nc.vector.tensor_scalar(
                    out=ht[:], in0=htp[:], scalar1=0.0, scalar2=None,
                    op0=mybir.AluOpType.max,
                )
            hT.append(ht)

        # fc2: out = h @ W2 + b2 (hT_j is already the lhsT layout)
        ones1 = sb.tile([1, BS], f16, tag="ones1")
        nc.vector.memset(ones1[:], 1.0)
        op_ = ps.tile([BS, O], f32, tag="op", space="PSUM")
        for j in range(4):
            nc.tensor.matmul(out=op_[:], lhsT=hT[j][:],
                             rhs=aux[:, AUX_W2[j]:AUX_W2[j] + O],
                             start=(j == 0), stop=False)
        nc.tensor.matmul(out=op_[:], lhsT=ones1[:],
                         rhs=aux[0:1, AUX_B2:AUX_B2 + O],
                         start=False, stop=True)
        # transpose the [128, 2] result to [2, 128] so the store is 2 records
        out_sb = sb.tile([BS, O], f32, tag="osb")
        nc.vector.tensor_copy(out=out_sb[:], in_=op_[:])
        otp = ps2.tile([O, BS], f32, tag="otp", space="PSUM")
        nc.tensor.transpose(out=otp[:], in_=out_sb[:], identity=ident[:])
        oT = sb.tile([O, BS], f32, tag="oT")
        nc.vector.tensor_copy(out=oT[:], in_=otp[:])
        nc.sync.dma_start(out=out_d[:], in_=oT[:])

    nc.finalize()
    return nc


def _prep_in_maps(text, lengths, emb_table, W1, b1, W2, b2):
    text = np.asarray(text).astype(np.int64)        # [S, B]
    lengths = np.asarray(lengths).astype(np.int64)  # [B]
    emb = np.asarray(emb_table, np.float32)
    w1b = np.vstack([np.asarray(W1, np.float32),
                     np.asarray(b1, np.float32)[None, :]]).astype(np.float16)
    w2 = np.asarray(W2, np.float32).astype(np.float16)
    b2f = np.asarray(b2, np.float32).astype(np.float16)

    # assign columns to cores greedily to minimize the max distinct-token
    # count per core (the slowest core's count sets every core's stream
    # length); kernel output is unpermuted on the host afterwards
    import os
    svec = np.arange(S)[:, None]
    col_tokens = [np.unique(text[:lengths[b], b]) for b in range(B)]
    order = np.argsort([-len(t) for t in col_tokens])
    if os.environ.get("KERNEL_NO_REBALANCE"):
        col_perm = np.arange(B)
        order = []
    seen = np.zeros((NCORES, V), bool)
    counts = [0] * NCORES
    dist = [0] * NCORES
    assign = [[] for _ in range(NCORES)]
    for b in order:
        toks = col_tokens[b]
        best, binc = None, None
        for i in range(NCORES):
            if counts[i] >= BS:
                continue
            inc = int(np.count_nonzero(~seen[i, toks]))
            # tie-break toward the emptier core to keep loads even
            key = (dist[i] + inc, counts[i])
            if best is None or key < bkey:
                best, binc, bkey = i, inc, key
        assign[best].append(b)
        counts[best] += 1
        dist[best] += binc
        seen[best, col_tokens[b]] = True
    if not os.environ.get("KERNEL_NO_REBALANCE"):
        col_perm = np.concatenate([np.sort(np.array(a, np.int64))
                                   for a in assign])

    colid = np.broadcast_to(np.arange(BS)[None, :], (S, BS))
    per_core = []
    cmax = 0
    for i in range(NCORES):
        cols = col_perm[i * BS:(i + 1) * BS]
        t_sh = text[:, cols]
        l_sh = lengths[cols]
        mask = svec < l_sh[None, :]
        used, inv = np.unique(t_sh[mask], return_inverse=True)
        cnt = np.zeros((len(used), BS), np.float32)
        np.add.at(cnt, (inv, colid[mask]), 1.0)
        cmax = max(cmax, cnt.max())
        per_core.append((used, cnt, l_sh))

    mode = "nib" if cmax <= 15 else ("u8" if cmax <= 255 else "f16")
    nmax = max(len(u) for u, _, _ in per_core)
    nct = -(-nmax // 128)
    plan, hc = _plan(nct, mode)
    ntiles = len(plan)

    in_maps = []
    for used, cnt, l_sh in per_core:
        n = len(used)
        if mode == "nib":
            a_pad = np.zeros((2 * hc * 128, BS), np.uint8)
            a_pad[:n] = cnt
            a_full = a_pad[:hc * 128] | (a_pad[hc * 128:] << 4)
        else:
            a_full = np.zeros((hc * 128, BS),
                              np.uint8 if mode == "u8" else np.float16)
            a_full[:n] = cnt
        a_w = (a_full.reshape(hc, 128, BS)
               .transpose(1, 0, 2).reshape(BS, hc * 128))
        e_full = np.zeros((nct * 128, E), np.float16)
        e_full[:n] = emb[used]
        e_w = np.zeros((ntiles * BS, G * EP), np.float16)
        for t, (c0, g, _) in enumerate(plan):
            ech = e_full[c0 * 128:(c0 + g) * 128].reshape(g, 128, E)
            e_w[t * BS:(t + 1) * BS, :g * EP] = (
                np.pad(ech, ((0, 0), (0, 0), (0, EP - E)))
                .transpose(1, 0, 2).reshape(BS, g * EP))
        aux = np.zeros((BS, AUXW), np.float16)
        for c, (r0, r1) in enumerate([(0, 128), (128, 256), (256, E + 1)]):
            aux[0:r1 - r0, c * 512:(c + 1) * 512] = w1b[r0:r1]
        for j in range(4):
            aux[:, AUX_W2[j]:AUX_W2[j] + O] = w2[j * 128:(j + 1) * 128]
        aux[0, AUX_B2:AUX_B2 + O] = b2f
        aux[:, AUX_LEN] = l_sh.astype(np.float16)  # lengths <= 512, exact
        in_maps.append({
            "aw": np.ascontiguousarray(a_w),
            "ew": np.ascontiguousarray(e_w),
            "aux": aux,
        })
    return in_maps, nct, mode, col_perm


def _run(inputs, trace=False):
    from concourse.bass_utils import run_bass_kernel_spmd

    in_maps, nct, mode, col_perm = _prep_in_maps(**inputs)
    nc = _build_nc(nct, mode)
    res = run_bass_kernel_spmd(nc, in_maps, list(range(NCORES)), trace=trace)
    perm_out = np.concatenate(
        [res.results[i]["out"].T for i in range(NCORES)], axis=0)
    out = np.empty_like(perm_out)
    out[col_perm] = perm_out
    return out.astype(np.float32), res


def kernel(**inputs):
    out, _ = _run(inputs, trace=False)
    return out
